# revision 1
# baseline (speedup 1.0000x reference)
"""Trainium2 Bass kernel: Mistral-style GQA attention with sliding-window mask.

Problem: hidden [1,2048,4096] -> Wq/Wk/Wv projections (32 q heads, 8 kv heads,
head_dim 128) -> RoPE -> sliding-window (1024) causal attention -> Wo.

Sharding: tensor-parallel over heads on 8 NeuronCores. Core i owns KV head i
and query heads 4i..4i+3 (Wq/Wk/Wv row-sharded, Wo column-sharded). Each core
computes partial_i = attn_heads_i @ Wo_i^T in HBM; host sums the 8 partials
(the TP all-reduce) to produce the full output.

On-device per core:
  phase A: stream H^T tiles from HBM, cast bf16, matmul into Q^T/K^T/V^T
           (layout [head_dim, seq]), fused RoPE on Q/K out of PSUM,
           V^T transposed to natural [seq, head_dim] via DMA-transpose.
  phase B: block-sparse attention per 512-query chunk: scores^T = K^T-block
           x Q^T-chunk on PE, exp on ACT (PSUM->SBUF bf16), static triangular
           masks on DVE, P@V and row-sum denominators on PE (ones-vector
           matmul), reciprocal+partition_broadcast for normalization, then
           Wo matmuls and fp32 output copy.
"""

import sys

for _p in ("/opt/trn_rl_repo", "/root/.axon_site/_ro/trn_rl_repo"):
    if _p not in sys.path:
        sys.path.insert(0, _p)

import numpy as np
import ml_dtypes

import concourse.bass as bass  # noqa: F401  (registers engine classes)
import concourse.mybir as mybir
import concourse.tile as tile
from concourse import bacc
from concourse.bass_utils import run_bass_kernel_spmd

S = 2048
HID = 4096
D = 128
NQH = 4          # query heads per core
NCORES = 8
SC = 512         # seq chunk
NCH = S // SC
KT = HID // 128  # contraction tiles
WINDOW = 1024
ROPE_BASE = 10000.0
SCALE = 1.0 / float(np.sqrt(D))

F32 = mybir.dt.float32
BF16 = mybir.dt.bfloat16
MULT = mybir.AluOpType.mult
ADD = mybir.AluOpType.add
SUB = mybir.AluOpType.subtract
EXP = mybir.ActivationFunctionType.Exp

# ptb slot layout: slot sl = kb - 4c + 8 for key-block kb in chunk c.
# exp-written region per slot, and statically-zero (memset once) regions.
def _slot_region(sl):
    lo = 128 * (sl - 8) if sl >= 8 else 0
    hi = 512 if sl >= 3 else 128 * (sl + 1)
    return lo, hi

_INVIS = []
for _sl in range(12):
    _lo, _hi = _slot_region(_sl)
    if _lo > 0:
        _INVIS.append((_sl, 0, _lo))
    if _hi < 512:
        _INVIS.append((_sl, _hi, 512))


def _program(tc, dr, out, niter=1, fused=True):
    nc = tc.nc
    ht, wqt, wkt, wvt, wot = dr["ht"], dr["wqt"], dr["wkt"], dr["wvt"], dr["wot"]
    ctab, stab, mcaus, mwin = dr["ctab"], dr["stab"], dr["mcaus"], dr["mwin"]

    def _copy(eng, out_ap, in_ap):
        if eng is nc.scalar:
            eng.copy(out_ap, in_ap)
        else:
            eng.tensor_copy(out_ap, in_ap)

    cast_engines = [nc.vector, nc.gpsimd]
    cast_idx = [0]

    def cast_rr(out_ap, in_ap):
        _copy(cast_engines[cast_idx[0] % 2], out_ap, in_ap)
        cast_idx[0] += 1

    # psum-reading copy engines (gpsimd cannot touch PSUM); ACT-heavy split
    pcopy_engines = [nc.scalar, nc.vector]
    pcopy_idx = [0]

    def pcopy_rr(out_ap, in_ap):
        _copy(pcopy_engines[pcopy_idx[0] % 2], out_ap, in_ap)
        pcopy_idx[0] += 1

    from contextlib import ExitStack
    for _it in range(niter):
        with ExitStack() as ctx:
            pw = ctx.enter_context(tc.tile_pool(name="persist", bufs=1))
            pst = ctx.enter_context(tc.tile_pool(name="stage", bufs=13))
            prt = ctx.enter_context(tc.tile_pool(name="ropet", bufs=2))

            wqb = pw.tile([128, KT * 512], BF16, name="wqb")
            wkb = pw.tile([128, KT * 128], BF16, name="wkb")
            wvb = pw.tile([128, KT * 128], BF16, name="wvb")
            wob = pw.tile([128, NQH * HID], BF16, name="wob")
            qtb = [pw.tile([128, S], BF16, name=f"qtb{h}") for h in range(NQH)]
            ktb = pw.tile([128, S], BF16, name="ktb")
            vtb = pw.tile([128, S], BF16, name="vtb")
            vnat = pw.tile([128, S], BF16, name="vnat")
            cs_t = pw.tile([128, S], F32, name="cs_t")
            sn_t = pw.tile([128, S], F32, name="sn_t")
            mc_t = pw.tile([128, 128], BF16, name="mc_t")
            mw_t = pw.tile([128, 128], BF16, name="mw_t")
            ones_t = pw.tile([128, 1], BF16, name="ones_t")

            # constants
            nc.sync.dma_start(cs_t[:], ctab[:])
            nc.sync.dma_start(sn_t[:], stab[:])
            nc.sync.dma_start(mc_t[:], mcaus[:])
            nc.sync.dma_start(mw_t[:], mwin[:])
            nc.gpsimd.memset(ones_t[:], 1.0)

            # rope helper state
            rope_state = [0]

            def _rope(dst, p, c):
                """dst[bf16 [128,512] slice] = rope(p [psum f32 [128,512]]) at chunk c.

                cs_t is cos duplicated across both halves; sn_t is sign-baked
                sin: rows 0:64 = -sin, rows 64:128 = +sin, so
                out = q*cos + rot(q)*sn with rot a plain half-swap.
                """
                use_gp = rope_state[0] % 5 >= 3
                rope_state[0] += 1
                csl = cs_t[:, SC * c:SC * (c + 1)]
                snl = sn_t[:, SC * c:SC * (c + 1)]
                if use_gp:
                    pre = prt.tile([128, 512], F32, tag="rpre", name="rpre", bufs=1)
                    rot = prt.tile([128, 512], F32, tag="rrot", name="rrot", bufs=1)
                    nc.scalar.copy(pre[:], p[:])
                    nc.scalar.copy(rot[0:64, :], p[64:128, :])
                    nc.scalar.copy(rot[64:128, :], p[0:64, :])
                    g1 = prt.tile([128, 512], F32, tag="rt1", name="g1")
                    g2 = prt.tile([128, 512], F32, tag="rt2", name="g2")
                    nc.gpsimd.tensor_tensor(g1[:], pre[:], csl, MULT)
                    nc.gpsimd.tensor_tensor(g2[:], rot[:], snl, MULT)
                    nc.gpsimd.tensor_tensor(dst[:, :], g1[:], g2[:], ADD)
                    return
                e = nc.vector
                t1 = prt.tile([64, 512], F32, tag="rt1", name="rt1")
                t2 = prt.tile([64, 512], F32, tag="rt2", name="rt2")
                e.tensor_tensor(t1[:], p[0:64, :], csl[0:64, :], MULT)
                e.tensor_tensor(t2[:], p[64:128, :], snl[0:64, :], MULT)
                e.tensor_tensor(dst[0:64, :], t1[:], t2[:], ADD)
                t3 = prt.tile([64, 512], F32, tag="rt1", name="rt3")
                t4 = prt.tile([64, 512], F32, tag="rt2", name="rt4")
                e.tensor_tensor(t3[:], p[64:128, :], csl[64:128, :], MULT)
                e.tensor_tensor(t4[:], p[0:64, :], snl[64:128, :], MULT)
                e.tensor_tensor(dst[64:128, :], t3[:], t4[:], ADD)

            phb = ctx.enter_context(tc.tile_pool(name="htbp", bufs=8))
            ppt = ctx.enter_context(tc.tile_pool(name="ptp", bufs=7))
            pmisc = ctx.enter_context(tc.tile_pool(name="miscb", bufs=2))
            pat = ctx.enter_context(tc.tile_pool(name="atbp", bufs=10))
            posb = ctx.enter_context(tc.tile_pool(name="osbp", bufs=4))

            # per-chunk pipeline: projections (+weight streaming on c==0),
            # then block-sparse attention, then Wo partial + output store.
            # fused=True interleaves the two stages per chunk; False runs all
            # projections first, then all attention chunks.
            def proj_stage(c):
                with tc.tile_pool(name="projps", bufs=6, space="PSUM") as ppp:
                    ps6 = [ppp.tile([128, 512], F32, tag="proj", name=f"proj{c}_{i}")
                           for i in range(6)]
                    for k in range(KT):
                        if c == 1:
                            # Wo weights are first needed when chunk 0's Wo is
                            # drained during attn(1); loading here keeps chunk 0
                            # (already DMA-heavy) lighter.
                            stwo = pst.tile([128, 512], F32, tag="stage", name="stwo")
                            nc.sync.dma_start(stwo[:], wot[128 * (k % 4):128 * (k % 4 + 1),
                                                           512 * (k // 4):512 * (k // 4 + 1)])
                            cast_rr(wob[:, HID * (k % 4) + 512 * (k // 4):
                                            HID * (k % 4) + 512 * (k // 4 + 1)], stwo[:])
                        if c == 0:
                            stw = pst.tile([128, 512], F32, tag="stage", name="stwq")
                            nc.sync.dma_start(stw[:], wqt[128 * k:128 * (k + 1), :])
                            cast_rr(wqb[:, 512 * k:512 * (k + 1)], stw[:])
                            stkv = pst.tile([128, 256], F32, tag="stage", name="stkv")
                            nc.sync.dma_start(stkv[:, 0:128], wkt[128 * k:128 * (k + 1), :])
                            nc.sync.dma_start(stkv[:, 128:256], wvt[128 * k:128 * (k + 1), :])
                            cast_rr(wkb[:, 128 * k:128 * (k + 1)], stkv[:, 0:128])
                            cast_rr(wvb[:, 128 * k:128 * (k + 1)], stkv[:, 128:256])
                        st = pst.tile([128, 512], F32, tag="stage", name="sth")
                        nc.sync.dma_start(st[:], ht[128 * k:128 * (k + 1), SC * c:SC * (c + 1)])
                        hb = phb.tile([128, 512], BF16, tag="htb", name="hb")
                        _copy(nc.scalar if k % 2 == 0 else nc.vector, hb[:], st[:])
                        first, last = k == 0, k == KT - 1
                        for h in range(NQH):
                            nc.tensor.matmul(ps6[h][:], wqb[:, 512 * k + 128 * h:512 * k + 128 * (h + 1)],
                                             hb[:], start=first, stop=last)
                        nc.tensor.matmul(ps6[4][:], wkb[:, 128 * k:128 * (k + 1)], hb[:],
                                         start=first, stop=last)
                        nc.tensor.matmul(ps6[5][:], wvb[:, 128 * k:128 * (k + 1)], hb[:],
                                         start=first, stop=last)
                    _rope(ktb[:, SC * c:SC * (c + 1)], ps6[4], c)
                    for h in range(NQH):
                        _rope(qtb[h][:, SC * c:SC * (c + 1)], ps6[h], c)
                    nc.scalar.copy(vtb[:, SC * c:SC * (c + 1)], ps6[5][:])
                    for b4 in range(4):
                        nc.sync.dma_start_transpose(
                            vnat[:, 128 * (4 * c + b4):128 * (4 * c + b4 + 1)],
                            vtb[:, SC * c + 128 * b4:SC * c + 128 * (b4 + 1)])

            def emit_wo_group(pop, wc, wj, wn, watbs):
                """One Wo output tile [128q, 512hid] for chunk wc: 4 head-MMs,
                PSUM->SBUF copy, store."""
                po = pop.tile([128, 512], F32, tag="po", name="po")
                for h in range(NQH):
                    nc.tensor.matmul(po[:], watbs[h][:, 128 * wj:128 * (wj + 1)],
                                     wob[:, HID * h + 512 * wn:HID * h + 512 * (wn + 1)],
                                     start=(h == 0), stop=(h == NQH - 1))
                ob = posb.tile([128, 512], F32, tag="osb", name="osb")
                pcopy_rr(ob[:], po[:])
                nc.sync.dma_start(out[SC * wc + 128 * wj:SC * wc + 128 * (wj + 1),
                                      512 * wn:512 * (wn + 1)], ob[:])

            def attn_stage(c, prev):
                # ---- attention for this chunk (past K/V only: sliding window),
                # with the PREVIOUS chunk's Wo matmul groups interleaved between
                # key-blocks so PE has independent work during exp waits.
                wo_pending = []
                if prev is not None:
                    pc, patbs = prev
                    wo_pending = [(pc, j, n, patbs) for j in range(4) for n in range(8)]
                with tc.tile_pool(name="scps", bufs=3, space="PSUM") as psc, \
                     tc.tile_pool(name="pvps", bufs=2, space="PSUM") as ppv, \
                     tc.tile_pool(name="denps", bufs=2, space="PSUM") as pdn, \
                     tc.tile_pool(name="outps", bufs=1, space="PSUM") as pop:
                    kbs = list(range(max(0, 4 * c - 8), 4 * c + 4))
                    first_kb, last_kb = kbs[0], kbs[-1]
                    # give PE independent work while DVE runs this chunk's rope
                    for _ in range(min(6, len(wo_pending))):
                        emit_wo_group(pop, *wo_pending.pop(0))
                    atbs = []
                    for h0 in range(0, NQH, 2):
                        # process a PAIR of heads per key-block sweep: two
                        # independent score/exp chains per step keep ACT fed.
                        pvs = [ppv.tile([128, 512], F32, tag="pv", name="pv")
                               for _ in range(2)]
                        dens = [pdn.tile([1, 512], F32, tag="den", name="den")
                                for _ in range(2)]

                        def emit_pv(kb, pts):
                            # accumulate P@V and row-sums over exact visible slices.
                            # start=True on the first key-block clears the bank's
                            # has_written bits; later blocks auto-overwrite elements
                            # they touch first and accumulate elsewhere.
                            sl = kb - 4 * c + 8
                            lo, hi = _slot_region(sl)
                            vsl = vnat[:, 128 * kb:128 * (kb + 1)]
                            for i in range(2):
                                nc.tensor.matmul(pvs[i][:, lo:hi], vsl, pts[i][:, lo:hi],
                                                 start=(kb == first_kb), stop=(kb == last_kb),
                                                 skip_group_check=True)
                                nc.tensor.matmul(dens[i][:, lo:hi], ones_t[:], pts[i][:, lo:hi],
                                                 start=(kb == first_kb), stop=(kb == last_kb),
                                                 skip_group_check=True)

                        prev = None
                        for kb in kbs:
                            sl = kb - 4 * c + 8
                            lo, hi = _slot_region(sl)
                            pts = []
                            for i in range(2):
                                sc = psc.tile([128, 512], F32, tag="sc", name="sc")
                                nc.tensor.matmul(sc[:, lo:hi], ktb[:, 128 * kb:128 * (kb + 1)],
                                                 qtb[h0 + i][:, SC * c + lo:SC * c + hi],
                                                 start=True, stop=True)
                                pt = ppt.tile([128, 512], BF16, tag="pt", name="pt")
                                nc.scalar.activation(pt[:, lo:hi], sc[:, lo:hi], EXP, scale=SCALE)
                                if sl <= 3:
                                    mofs = 128 * sl
                                    nc.vector.tensor_tensor(pt[:, mofs:mofs + 128],
                                                            pt[:, mofs:mofs + 128], mw_t[:], MULT)
                                elif sl >= 8:
                                    mofs = 128 * (sl - 8)
                                    nc.vector.tensor_tensor(pt[:, mofs:mofs + 128],
                                                            pt[:, mofs:mofs + 128], mc_t[:], MULT)
                                pts.append(pt)
                            if prev is not None:
                                emit_pv(*prev)
                            if wo_pending:
                                emit_wo_group(pop, *wo_pending.pop(0))
                            prev = (kb, pts)
                        emit_pv(*prev)
                        if wo_pending:
                            emit_wo_group(pop, *wo_pending.pop(0))
                        for i in range(2):
                            dre = pmisc.tile([1, 512], F32, tag="denr", name="denr")
                            nc.vector.reciprocal(dre[:], dens[i][:])
                            dbc = pmisc.tile([128, 512], F32, tag="denb", name="denb")
                            nc.gpsimd.partition_broadcast(dbc[:], dre[:])
                            at = pat.tile([128, 512], BF16, tag="atb", name="atb")
                            nc.vector.tensor_tensor(at[:], pvs[i][:], dbc[:], MULT)
                            atbs.append(at)
                    while wo_pending:
                        emit_wo_group(pop, *wo_pending.pop(0))
                return atbs

            prev = None
            if fused:
                for c in range(NCH):
                    proj_stage(c)
                    prev = (c, attn_stage(c, prev))
            else:
                for c in range(NCH):
                    proj_stage(c)
                for c in range(NCH):
                    prev = (c, attn_stage(c, prev))
            # final chunk's Wo tail
            with tc.tile_pool(name="outps", bufs=2, space="PSUM") as pop:
                pc, patbs = prev
                for j in range(4):
                    for n in range(8):
                        emit_wo_group(pop, pc, j, n, patbs)


_NC_CACHE = {}


def _build(niter=1, fused=True):
    import os
    fused = os.environ.get("KERNEL_FUSED", "1" if fused else "0") == "1"
    key = (niter, fused)
    if key in _NC_CACHE:
        return _NC_CACHE[key]
    nc = bacc.Bacc("TRN2", target_bir_lowering=False, debug=False,
                   enable_asserts=True, num_devices=NCORES)
    dr = {}

    def din(name, shape, dt=F32):
        dr[name] = nc.dram_tensor(name, shape, dt, kind="ExternalInput").ap()

    din("ht", [HID, S])
    din("wqt", [HID, NQH * D])
    din("wkt", [HID, D])
    din("wvt", [HID, D])
    din("wot", [NQH * D, HID])
    din("ctab", [128, S])
    din("stab", [128, S])
    din("mcaus", [128, 128], BF16)
    din("mwin", [128, 128], BF16)
    out = nc.dram_tensor("out", [S, HID], F32, kind="ExternalOutput").ap()

    with tile.TileContext(nc) as tc:
        _program(tc, dr, out, niter, fused)
    nc.compile()
    _NC_CACHE[key] = nc
    return nc


def make_in_maps(inputs):
    hs = np.asarray(inputs["hidden_states"], dtype=np.float32)
    Wq = np.asarray(inputs["Wq"], dtype=np.float32)
    Wk = np.asarray(inputs["Wk"], dtype=np.float32)
    Wv = np.asarray(inputs["Wv"], dtype=np.float32)
    Wo = np.asarray(inputs["Wo"], dtype=np.float32)
    pos = np.asarray(inputs["position_ids"]).reshape(-1)

    assert hs.shape == (1, S, HID), hs.shape
    H = hs[0]
    HT = np.ascontiguousarray(H.T)

    # RoPE tables in [d%64, s] layout (fp32, mirroring the reference math)
    inv = (1.0 / (ROPE_BASE ** (np.arange(0, D, 2, dtype=np.float32) / D))).astype(np.float32)
    ang = pos.astype(np.float32)[None, :] * inv[:, None]          # [64, S]
    cos64 = np.cos(ang).astype(np.float32)
    sin64 = np.sin(ang).astype(np.float32)
    ctab = np.concatenate([cos64, cos64], axis=0)                 # [128, S]
    stab = np.concatenate([-sin64, sin64], axis=0)                # sign-baked

    kk = np.arange(128)[:, None]
    qq = np.arange(128)[None, :]
    mcaus = (qq >= kk).astype(ml_dtypes.bfloat16)   # causal diag block, [k,q]
    mwin = (qq < kk).astype(ml_dtypes.bfloat16)     # window-edge block, [k,q]

    in_maps = []
    for i in range(NCORES):
        in_maps.append({
            "ht": HT,
            "wqt": np.ascontiguousarray(Wq[512 * i:512 * (i + 1), :].T),
            "wkt": np.ascontiguousarray(Wk[128 * i:128 * (i + 1), :].T),
            "wvt": np.ascontiguousarray(Wv[128 * i:128 * (i + 1), :].T),
            "wot": np.ascontiguousarray(Wo[:, 512 * i:512 * (i + 1)].T),
            "ctab": ctab,
            "stab": stab,
            "mcaus": mcaus,
            "mwin": mwin,
        })

    return in_maps


def kernel(**inputs):
    in_maps = make_in_maps(inputs)
    nc = _build()
    res = run_bass_kernel_spmd(nc, in_maps, core_ids=list(range(NCORES)))

    acc = np.zeros((S, HID), dtype=np.float64)
    for r in res.results:
        acc += r["out"].astype(np.float64)
    return acc.astype(np.float32).reshape(1, S, HID)

